# revision 32
# baseline (speedup 1.0000x reference)
"""Trainium2 Bass kernel for nn_LBLHighwayBiLmV2 (bf16 compute).

Computation (per batch element b, per layer l in {0,1}):
    fwd = band_fwd(fwd); bwd = band_bwd(bwd)          # 17-tap causal banded mix along L
    fwd = highway(fwd, fW[l], fb[l])  (2 steps)       # x = g*x + (1-g)*relu(nonlin)
    bwd = highway(bwd, bW[l], bb[l])
    if l: fwd += f_cache; bwd += b_cache
    out[l, b] = concat(fwd, bwd, axis=-1)

Sharding: data-parallel over batch, 1 sample per NeuronCore (8 cores).

Per-core dataflow (bf16 operands, fp32 PSUM accumulate):
  - "home" layout for activations is transposed: vT = [H=512, L=2048] as 4 SBUF
    tiles [128, 2048] (feature chunks on partitions).
  - band: PE matmuls y^T[d,t] = sum_s x_nat[s,d] * S[s,t], with x natural
    (streamed from DRAM for layer 0, resident SBUF tiles for layer 1) as lhsT
    and precomputed banded coefficient blocks (host-built, tiny) as rhs.
  - highway matmuls: lhsT = W^T chunks (host-pre-transposed), rhs = vT -> PSUM;
    gate half evacuated by ACT with fused bias+sigmoid, nonlinear half by DVE
    tensor_scalar with fused bias-add + max(0).
  - gating on DVE: z = r + g*(v - r)  (3 tensor_tensor ops per chunk).
  - output transpose back to natural layout on PE (128x128 blocks into PSUM),
    evacuated ACT/DVE, DMA'd to DRAM as bf16 (host casts to fp32).
"""

import os
import sys
from contextlib import ExitStack

import numpy as np

sys.path.insert(0, "/opt/trn_rl_repo")

import concourse.bacc as bacc
import concourse.bass as bass
import concourse.mybir as mybir
import concourse.tile as tile
from concourse.masks import make_identity

F32 = mybir.dt.float32
BF16 = mybir.dt.bfloat16
NP_BF16 = mybir.dt.np(BF16)

N_LAYERS = 2
N_HW = 2
B, L, H = 8, 2048, 512
WIDTH = 16
TWO_H = 2 * H
NT = L // 128      # 16 sequence tiles
ND = H // 128      # 4 feature chunks
NJ = L // 512      # 4 t-blocks of 512


# ----------------------------------------------------------------------------
# Host-side parameter prep (tiny, batch-independent)
# ----------------------------------------------------------------------------

def _fwd_blocks(w):
    """w: [17] logits (torch slicing convention). Returns (D_pad[128,1024],
    D_first[128,512]) fp32 blocks for the forward band matmuls.

    Forward score: S[s,t] = exp(w[16-(t-s)]) / Z_t for 0 <= t-s <= 16,
    Z_t = sum over valid d of exp(w[16-d]) (d = 0..min(16,t))."""
    w = np.asarray(w, np.float64)
    e = np.exp(w[16 - np.arange(17)] - w.max())      # e[d] = exp(w[16-d]) scaled
    cf = (e / e.sum()).astype(np.float64)            # full-column weights

    s = np.arange(128)[:, None]
    c = np.arange(1024)[None, :]
    d = (c - 384) - s
    Dp = np.where((d >= 0) & (d <= 16), cf[np.clip(d, 0, 16)], 0.0)

    t = np.arange(512)[None, :]
    d2 = t - s
    Zt = np.cumsum(e)[np.minimum(t, 16)]             # truncated norm for t<16
    Df = np.where((d2 >= 0) & (d2 <= 16), e[np.clip(d2, 0, 16)] / Zt, 0.0)
    return Dp.astype(np.float32), Df.astype(np.float32)


def _bwd_blocks(w):
    """Backward: S[s,t] = exp(w[s-t]) / Z_t for 0 <= s-t <= 16,
    Z_t truncated for t > L-17. Returns (E_pad[128,1024], E_last[128,512])."""
    w = np.asarray(w, np.float64)
    e = np.exp(w - w.max())                          # e[d] = exp(w[d]) scaled
    cb = (e / e.sum()).astype(np.float64)

    s = np.arange(128)[:, None]
    c = np.arange(1024)[None, :]
    d = s - (c - 512)
    Ep = np.where((d >= 0) & (d <= 16), cb[np.clip(d, 0, 16)], 0.0)

    sg = 1920 + s
    tg = 1536 + np.arange(512)[None, :]
    d2 = sg - tg
    lim = np.minimum(16, (L - 1) - tg)
    Zt = np.cumsum(e)[np.clip(lim, 0, 16)]
    El = np.where((d2 >= 0) & (d2 <= lim), e[np.clip(d2, 0, 16)] / Zt, 0.0)
    return Ep.astype(np.float32), El.astype(np.float32)


def _prep_params(f_scores, b_scores, fW, fb, bW, bb):
    bandf = np.zeros((N_LAYERS, 128, 1536), np.float32)
    bandb = np.zeros((N_LAYERS, 128, 1536), np.float32)
    for l in range(N_LAYERS):
        Dp, Df = _fwd_blocks(f_scores[l])
        bandf[l, :, :1024] = Dp
        bandf[l, :, 1024:] = Df
        Ep, El = _bwd_blocks(b_scores[l])
        bandb[l, :, :1024] = Ep
        bandb[l, :, 1024:] = El

    # wt[l, dir, h, d, o] = W[l?,dir,h][o, d]  (torch Linear weight transposed)
    # bias packed directly in SBUF layout: [128, (l d h oc)] with
    # bias[p, ((l*2+d)*2+h)*8+oc] = b[l,d,h][oc*128+p]
    wt = np.zeros((N_LAYERS, 2, N_HW, H, TWO_H), np.float32)
    bias = np.zeros((128, N_LAYERS * 2 * N_HW * 8), np.float32)
    for l in range(N_LAYERS):
        for d_, Wsrc, bsrc in ((0, fW, fb), (1, bW, bb)):
            for h in range(N_HW):
                wt[l, d_, h] = np.ascontiguousarray(np.asarray(Wsrc[l, h]).T)
                col0 = ((l * 2 + d_) * N_HW + h) * 8
                bias[:, col0:col0 + 8] = (
                    np.asarray(bsrc[l, h]).reshape(TWO_H // 128, 128).T)
    return (bandf.astype(NP_BF16), bandb.astype(NP_BF16),
            wt.astype(NP_BF16), bias)


def build_in_maps(inputs, f_scores, b_scores, fW, fb, bW, bb):
    """Per-core input dicts for run_bass_kernel_spmd (bf16 payloads)."""
    bandf, bandb, wt, bias = _prep_params(
        np.asarray(f_scores), np.asarray(b_scores),
        np.asarray(fW), np.asarray(fb), np.asarray(bW), np.asarray(bb))
    x16 = np.ascontiguousarray(np.asarray(inputs, np.float32)).astype(NP_BF16)
    return [
        {"x": x16[b], "wt": wt, "bias": bias, "bandf": bandf, "bandb": bandb}
        for b in range(B)
    ]


# ----------------------------------------------------------------------------
# Bass kernel
# ----------------------------------------------------------------------------

def _fwd_members(j, Dp, Df):
    """(s-tile, rhs AP, lo, hi): column-sliced matmuls for fwd band t-block j.

    s-tile i covers t in [128i, 128i+143]; within block j only the covered
    column span is streamed (PSUM has_written handles the 16-col overlaps)."""
    out = []
    if j == 0:
        out.append((0, Df[:, 0:144], 0, 144))
    else:
        out.append((4 * j - 1, Dp[:, 512:528], 0, 16))
    for m in range(0 if j else 1, 4):
        lo, hi = 128 * m, min(512, 128 * m + 144)
        out.append((4 * j + m, Dp[:, 384 - 128 * m + lo:384 - 128 * m + hi],
                    lo, hi))
    return out


def _bwd_members(j, Ep, El):
    """Backward: s-tile i covers t in [128i-16, 128i+127]."""
    out = []
    for m in (0, 1, 2, 3, 4):
        i = 4 * j + m
        if i >= NT:
            continue
        lo, hi = max(0, 128 * m - 16), min(512, 128 * m + 128)
        if lo >= hi:
            continue
        if j == NJ - 1 and m == 3:
            out.append((i, El[:, lo:hi], lo, hi))
        else:
            out.append((i, Ep[:, 512 - 128 * m + lo:512 - 128 * m + hi], lo, hi))
    return out


def build_nc():
    nc = bacc.Bacc("TRN2", target_bir_lowering=False, debug=False)

    x_ap = nc.dram_tensor("x", [L, H], BF16, kind="ExternalInput").ap()
    wt_ap = nc.dram_tensor("wt", [N_LAYERS, 2, N_HW, H, TWO_H], BF16,
                           kind="ExternalInput").ap()
    bias_ap = nc.dram_tensor("bias", [128, N_LAYERS * 2 * N_HW * 8], F32,
                             kind="ExternalInput").ap()
    bandf_ap = nc.dram_tensor("bandf", [N_LAYERS, 128, 1536], BF16,
                              kind="ExternalInput").ap()
    bandb_ap = nc.dram_tensor("bandb", [N_LAYERS, 128, 1536], BF16,
                              kind="ExternalInput").ap()
    out_ap = nc.dram_tensor("out", [N_LAYERS, L, TWO_H], BF16,
                            kind="ExternalOutput").ap()

    with tile.TileContext(nc) as tc, ExitStack() as ctx:
        const_pool = ctx.enter_context(tc.tile_pool(name="const", bufs=1))
        band_pool = ctx.enter_context(tc.tile_pool(name="band", bufs=2))
        wt_pool = ctx.enter_context(tc.tile_pool(name="wtp", bufs=8))
        xnat_pool = ctx.enter_context(tc.tile_pool(name="xnat", bufs=1))
        act_pool = ctx.enter_context(tc.tile_pool(name="act", bufs=1))
        rg_pool = ctx.enter_context(tc.tile_pool(name="rg", bufs=4))
        znat_pool = ctx.enter_context(tc.tile_pool(name="znat", bufs=1))
        mm_psum = ctx.enter_context(tc.tile_pool(name="mmp", bufs=4, space="PSUM"))

        bias_sb = const_pool.tile([128, N_LAYERS * 2 * N_HW * 8], F32, tag="bias")
        nc.sync.dma_start(bias_sb[:], bias_ap)

        # persistent activation tiles per direction, [128, 2048] per feature
        # chunk (both direction chains live concurrently for PE gap-filling)
        A = [[act_pool.tile([128, L], BF16, tag=f"A{d}{c}", name=f"A{d}{c}")
              for c in range(ND)] for d in (0, 1)]
        Bt = [[act_pool.tile([128, L], BF16, tag=f"B{d}{c}", name=f"B{d}{c}")
               for c in range(ND)] for d in (0, 1)]
        C = [[act_pool.tile([128, L], BF16, tag=f"C{d}{c}", name=f"C{d}{c}")
              for c in range(ND)] for d in (0, 1)]
        # natural-layout activations, dc-major so each chunk's xbar-transpose
        # destination is per-partition contiguous: zna[p, dc*L + k*128 + f]
        # = z_nat[k*128 + p, dc*128 + f]
        zna = [znat_pool.tile([128, ND * L], BF16, tag=f"zna{d}", name=f"zna{d}")
               for d in (0, 1)]
        # x resident in natural layout: layer-0 band lhsT for both directions
        xnat = [xnat_pool.tile([128, H], BF16, tag=f"xn{k}", name=f"xn{k}")
                for k in range(NT)]
        for k in range(NT):
            nc.sync.dma_start(xnat[k][:], x_ap[k * 128:(k + 1) * 128, :])

        def band(layer, dir_, dst):
            """dst: list of 4 chunk tiles [128, 2048] receiving y^T."""
            bsrc = bandf_ap if dir_ == 0 else bandb_ap
            bsb = band_pool.tile([128, 1536], BF16, tag="bandblk")
            nc.sync.dma_start(bsb[:], bsrc[layer])
            Pad = bsb[:, 0:1024]
            Spc = bsb[:, 1024:1536]
            members = _fwd_members if dir_ == 0 else _bwd_members
            if layer == 0:
                def src_ap(i, dc):
                    return xnat[i][:, dc * 128:(dc + 1) * 128]
            else:
                zd = zna[dir_]

                def src_ap(i, dc):
                    return zd[:, dc * L + i * 128:dc * L + (i + 1) * 128]
            for jp in range(NJ // 2):
                for dc in range(ND):
                    # [128,1024] PSUM tile spans 2 banks = 2 adjacent t-blocks;
                    # one evacuation instruction covers both
                    ps = mm_psum.tile([128, 1024], F32, tag="mm")
                    for half in (0, 1):
                        j = 2 * jp + half
                        mem = members(j, Pad, Spc)
                        n = len(mem)
                        for k, (i, rhs, lo, hi) in enumerate(mem):
                            nc.tensor.matmul(
                                ps[:, half * 512 + lo:half * 512 + hi],
                                src_ap(i, dc),
                                rhs,
                                start=(k == 0),
                                stop=(k == n - 1),
                            )
                    dstap = dst[dc][:, jp * 1024:(jp + 1) * 1024]
                    nc.scalar.copy(dstap, ps[:])

        def highway(layer, dir_, h, v, dst, resid=None):
            """v: input chunk tiles (transposed layout); dst: output chunk tiles.
            z = r + g*(v - r), r = relu(nl + bn), g = sigmoid(gt + bg)."""
            wts = []
            for dc in range(ND):
                wtile = wt_pool.tile([128, TWO_H], BF16, tag="wt")
                nc.sync.dma_start(
                    wtile[:], wt_ap[layer, dir_, h, dc * 128:(dc + 1) * 128, :])
                wts.append(wtile)
            bcol0 = ((layer * 2 + dir_) * N_HW + h) * 8
            # jp outer: all 4 chunks' half-row jp finish before jp+1 starts, so
            # the next stage's accumulation matmuls (which need every chunk)
            # unblock after half the gating instead of all of it
            for jp in range(NJ // 2):
                sl = slice(jp * 1024, (jp + 1) * 1024)
                for c in range(ND):
                    r = rg_pool.tile([128, 1024], BF16, tag="r")
                    g = rg_pool.tile([128, 1024], BF16, tag="g")
                    for part, oc in ((0, c), (1, 4 + c)):
                        ps = mm_psum.tile([128, 1024], F32, tag="mm")
                        for half in (0, 1):
                            j = 2 * jp + half
                            for dc in range(ND):
                                nc.tensor.matmul(
                                    ps[:, half * 512:(half + 1) * 512],
                                    wts[dc][:, oc * 128:(oc + 1) * 128],
                                    v[dc][:, j * 512:(j + 1) * 512],
                                    start=(dc == 0),
                                    stop=(dc == ND - 1),
                                )
                        bap = bias_sb[:, bcol0 + oc:bcol0 + oc + 1]
                        tgt = (r if part == 0 else g)[:]
                        if part == 0:
                            # r = max(psum + bias, 0) fused on DVE
                            nc.vector.tensor_scalar(
                                tgt, ps[:], bap, 0.0,
                                mybir.AluOpType.add, mybir.AluOpType.max)
                        else:
                            nc.scalar.activation(
                                tgt, ps[:],
                                mybir.ActivationFunctionType.Sigmoid, bias=bap)
                    # gate this half-row immediately: z = r + g*(v - r)
                    tmp = rg_pool.tile([128, 1024], BF16, tag="tmp")
                    nc.vector.tensor_sub(tmp[:], v[c][:, sl], r[:])
                    nc.vector.tensor_mul(tmp[:], tmp[:], g[:])
                    nc.vector.tensor_add(dst[c][:, sl], tmp[:], r[:])
                    if resid is not None:
                        nc.vector.tensor_add(
                            dst[c][:, sl], dst[c][:, sl], resid[c][:, sl])

        def transpose_out(layer, dir_, src, keep_nat):
            """src: 4 chunk tiles [128,2048] (transposed). xbar-DMA-transposes
            each chunk into zna[dir_] (natural layout, dc-major), then DMAs the
            natural data to DRAM out[layer, :, dir_*H:(dir_+1)*H]. zna doubles
            as the next layer's band input (layer-1 overwrite is WAR-safe)."""
            zd = zna[dir_]
            dram3 = out_ap[layer].rearrange("(k p) f -> p k f", p=128)
            # split per (chunk, half-row): each piece's gating finishes early
            # under the jp-outer highway loop, so these DMAs overlap compute
            for hb in range(2):
                for dc in range(ND):
                    off = dc * L + hb * 1024
                    dst3 = zd[:, off:off + 1024].rearrange(
                        "p (k f) -> p k f", f=128)
                    nc.sync.dma_start_transpose(
                        dst3, src[dc][:, hb * 1024:(hb + 1) * 1024])
                    nc.sync.dma_start(
                        dram3[:, hb * 8:(hb + 1) * 8,
                              dir_ * H + dc * 128:dir_ * H + (dc + 1) * 128],
                        dst3)

        # emit both directions interleaved per stage: the two chains are
        # independent, so whenever one stalls on its gating/evac tail the
        # scheduler has ready matmuls from the other to keep PE warm
        for d in (0, 1):
            band(0, d, A[d])
        for d in (0, 1):
            highway(0, d, 0, A[d], Bt[d])
        for d in (0, 1):
            highway(0, d, 1, Bt[d], C[d])
        for d in (0, 1):
            transpose_out(0, d, C[d], keep_nat=True)
        for d in (0, 1):
            band(1, d, A[d])
        for d in (0, 1):
            highway(1, d, 0, A[d], Bt[d])
        for d in (0, 1):
            highway(1, d, 1, Bt[d], A[d], resid=C[d])
        for d in (0, 1):
            transpose_out(1, d, A[d], keep_nat=False)

    nc.compile()
    return nc


_NC_CACHE = None
LAST_RESULTS = None


def _get_nc():
    global _NC_CACHE
    if _NC_CACHE is None:
        _NC_CACHE = build_nc()
    return _NC_CACHE


def kernel(inputs, masks, f_scores, b_scores, fW, fb, bW, bb):
    global LAST_RESULTS
    from concourse.bass_utils import run_bass_kernel_spmd

    in_maps = build_in_maps(inputs, f_scores, b_scores, fW, fb, bW, bb)
    nc = _get_nc()
    res = run_bass_kernel_spmd(nc, in_maps, core_ids=list(range(B)),
                               trace=bool(os.environ.get("BASS_TRACE")))
    LAST_RESULTS = res
    out = np.stack([res.results[b]["out"].astype(np.float32) for b in range(B)],
                   axis=1)
    return out


# revision 33
# speedup vs baseline: 1.1816x; 1.1816x over previous
"""Trainium2 Bass kernel for nn_LBLHighwayBiLmV2 (bf16 compute).

Computation (per batch element b, per layer l in {0,1}):
    fwd = band_fwd(fwd); bwd = band_bwd(bwd)          # 17-tap causal banded mix along L
    fwd = highway(fwd, fW[l], fb[l])  (2 steps)       # x = g*x + (1-g)*relu(nonlin)
    bwd = highway(bwd, bW[l], bb[l])
    if l: fwd += f_cache; bwd += b_cache
    out[l, b] = concat(fwd, bwd, axis=-1)

Sharding: data-parallel over batch, 1 sample per NeuronCore (8 cores).

Per-core dataflow (bf16 operands, fp32 PSUM accumulate):
  - "home" layout for activations is transposed: vT = [H=512, L=2048] as 4 SBUF
    tiles [128, 2048] (feature chunks on partitions).
  - band: PE matmuls y^T[d,t] = sum_s x_nat[s,d] * S[s,t], with x natural
    (streamed from DRAM for layer 0, resident SBUF tiles for layer 1) as lhsT
    and precomputed banded coefficient blocks (host-built, tiny) as rhs.
  - highway matmuls: lhsT = W^T chunks (host-pre-transposed), rhs = vT -> PSUM;
    gate half evacuated by ACT with fused bias+sigmoid, nonlinear half by DVE
    tensor_scalar with fused bias-add + max(0).
  - gating on DVE: z = r + g*(v - r)  (3 tensor_tensor ops per chunk).
  - output transpose back to natural layout on PE (128x128 blocks into PSUM),
    evacuated ACT/DVE, DMA'd to DRAM as bf16 (host casts to fp32).
"""

import os
import sys
from contextlib import ExitStack

import numpy as np

sys.path.insert(0, "/opt/trn_rl_repo")

import concourse.bacc as bacc
import concourse.bass as bass
import concourse.mybir as mybir
import concourse.tile as tile
from concourse.masks import make_identity

F32 = mybir.dt.float32
BF16 = mybir.dt.bfloat16
NP_BF16 = mybir.dt.np(BF16)

N_LAYERS = 2
N_HW = 2
B, L, H = 8, 2048, 512
WIDTH = 16
TWO_H = 2 * H
NT = L // 128      # 16 sequence tiles
ND = H // 128      # 4 feature chunks
NJ = L // 512      # 4 t-blocks of 512


# ----------------------------------------------------------------------------
# Host-side parameter prep (tiny, batch-independent)
# ----------------------------------------------------------------------------

def _fwd_blocks(w):
    """w: [17] logits (torch slicing convention). Returns (D_pad[128,1024],
    D_first[128,512]) fp32 blocks for the forward band matmuls.

    Forward score: S[s,t] = exp(w[16-(t-s)]) / Z_t for 0 <= t-s <= 16,
    Z_t = sum over valid d of exp(w[16-d]) (d = 0..min(16,t))."""
    w = np.asarray(w, np.float64)
    e = np.exp(w[16 - np.arange(17)] - w.max())      # e[d] = exp(w[16-d]) scaled
    cf = (e / e.sum()).astype(np.float64)            # full-column weights

    s = np.arange(128)[:, None]
    c = np.arange(1024)[None, :]
    d = (c - 384) - s
    Dp = np.where((d >= 0) & (d <= 16), cf[np.clip(d, 0, 16)], 0.0)

    t = np.arange(512)[None, :]
    d2 = t - s
    Zt = np.cumsum(e)[np.minimum(t, 16)]             # truncated norm for t<16
    Df = np.where((d2 >= 0) & (d2 <= 16), e[np.clip(d2, 0, 16)] / Zt, 0.0)
    return Dp.astype(np.float32), Df.astype(np.float32)


def _bwd_blocks(w):
    """Backward: S[s,t] = exp(w[s-t]) / Z_t for 0 <= s-t <= 16,
    Z_t truncated for t > L-17. Returns (E_pad[128,1024], E_last[128,512])."""
    w = np.asarray(w, np.float64)
    e = np.exp(w - w.max())                          # e[d] = exp(w[d]) scaled
    cb = (e / e.sum()).astype(np.float64)

    s = np.arange(128)[:, None]
    c = np.arange(1024)[None, :]
    d = s - (c - 512)
    Ep = np.where((d >= 0) & (d <= 16), cb[np.clip(d, 0, 16)], 0.0)

    sg = 1920 + s
    tg = 1536 + np.arange(512)[None, :]
    d2 = sg - tg
    lim = np.minimum(16, (L - 1) - tg)
    Zt = np.cumsum(e)[np.clip(lim, 0, 16)]
    El = np.where((d2 >= 0) & (d2 <= lim), e[np.clip(d2, 0, 16)] / Zt, 0.0)
    return Ep.astype(np.float32), El.astype(np.float32)


def _prep_params(f_scores, b_scores, fW, fb, bW, bb):
    bandf = np.zeros((N_LAYERS, 128, 1536), np.float32)
    bandb = np.zeros((N_LAYERS, 128, 1536), np.float32)
    for l in range(N_LAYERS):
        Dp, Df = _fwd_blocks(f_scores[l])
        bandf[l, :, :1024] = Dp
        bandf[l, :, 1024:] = Df
        Ep, El = _bwd_blocks(b_scores[l])
        bandb[l, :, :1024] = Ep
        bandb[l, :, 1024:] = El

    # wt[l, dir, h, d, o] = W[l?,dir,h][o, d]  (torch Linear weight transposed)
    # bias packed directly in SBUF layout: [128, (l d h oc)] with
    # bias[p, ((l*2+d)*2+h)*8+oc] = b[l,d,h][oc*128+p]
    wt = np.zeros((N_LAYERS, 2, N_HW, H, TWO_H), np.float32)
    bias = np.zeros((128, N_LAYERS * 2 * N_HW * 8), np.float32)
    for l in range(N_LAYERS):
        for d_, Wsrc, bsrc in ((0, fW, fb), (1, bW, bb)):
            for h in range(N_HW):
                wt[l, d_, h] = np.ascontiguousarray(np.asarray(Wsrc[l, h]).T)
                col0 = ((l * 2 + d_) * N_HW + h) * 8
                bias[:, col0:col0 + 8] = (
                    np.asarray(bsrc[l, h]).reshape(TWO_H // 128, 128).T)
    return (bandf.astype(NP_BF16), bandb.astype(NP_BF16),
            wt.astype(NP_BF16), bias)


def build_in_maps(inputs, f_scores, b_scores, fW, fb, bW, bb):
    """Per-core input dicts for run_bass_kernel_spmd (bf16 payloads)."""
    bandf, bandb, wt, bias = _prep_params(
        np.asarray(f_scores), np.asarray(b_scores),
        np.asarray(fW), np.asarray(fb), np.asarray(bW), np.asarray(bb))
    x16 = np.ascontiguousarray(np.asarray(inputs, np.float32)).astype(NP_BF16)
    return [
        {"x": x16[b], "wt": wt, "bias": bias, "bandf": bandf, "bandb": bandb}
        for b in range(B)
    ]


# ----------------------------------------------------------------------------
# Bass kernel
# ----------------------------------------------------------------------------

def _fwd_members(j, Dp, Df):
    """(s-tile, rhs AP, lo, hi): column-sliced matmuls for fwd band t-block j.

    s-tile i covers t in [128i, 128i+143]; within block j only the covered
    column span is streamed (PSUM has_written handles the 16-col overlaps)."""
    out = []
    if j == 0:
        out.append((0, Df[:, 0:144], 0, 144))
    else:
        out.append((4 * j - 1, Dp[:, 512:528], 0, 16))
    for m in range(0 if j else 1, 4):
        lo, hi = 128 * m, min(512, 128 * m + 144)
        out.append((4 * j + m, Dp[:, 384 - 128 * m + lo:384 - 128 * m + hi],
                    lo, hi))
    return out


def _bwd_members(j, Ep, El):
    """Backward: s-tile i covers t in [128i-16, 128i+127]."""
    out = []
    for m in (0, 1, 2, 3, 4):
        i = 4 * j + m
        if i >= NT:
            continue
        lo, hi = max(0, 128 * m - 16), min(512, 128 * m + 128)
        if lo >= hi:
            continue
        if j == NJ - 1 and m == 3:
            out.append((i, El[:, lo:hi], lo, hi))
        else:
            out.append((i, Ep[:, 512 - 128 * m + lo:512 - 128 * m + hi], lo, hi))
    return out


def build_nc():
    nc = bacc.Bacc("TRN2", target_bir_lowering=False, debug=False)

    x_ap = nc.dram_tensor("x", [L, H], BF16, kind="ExternalInput").ap()
    wt_ap = nc.dram_tensor("wt", [N_LAYERS, 2, N_HW, H, TWO_H], BF16,
                           kind="ExternalInput").ap()
    bias_ap = nc.dram_tensor("bias", [128, N_LAYERS * 2 * N_HW * 8], F32,
                             kind="ExternalInput").ap()
    bandf_ap = nc.dram_tensor("bandf", [N_LAYERS, 128, 1536], BF16,
                              kind="ExternalInput").ap()
    bandb_ap = nc.dram_tensor("bandb", [N_LAYERS, 128, 1536], BF16,
                              kind="ExternalInput").ap()
    out_ap = nc.dram_tensor("out", [N_LAYERS, L, TWO_H], BF16,
                            kind="ExternalOutput").ap()

    with tile.TileContext(nc) as tc, ExitStack() as ctx:
        const_pool = ctx.enter_context(tc.tile_pool(name="const", bufs=1))
        band_pool = ctx.enter_context(tc.tile_pool(name="band", bufs=2))
        wt_pool = ctx.enter_context(tc.tile_pool(name="wtp", bufs=8))
        xnat_pool = ctx.enter_context(tc.tile_pool(name="xnat", bufs=1))
        act_pool = ctx.enter_context(tc.tile_pool(name="act", bufs=1))
        rg_pool = ctx.enter_context(tc.tile_pool(name="rg", bufs=4))
        znat_pool = ctx.enter_context(tc.tile_pool(name="znat", bufs=1))
        mm_psum = ctx.enter_context(tc.tile_pool(name="mmp", bufs=4, space="PSUM"))

        bias_sb = const_pool.tile([128, N_LAYERS * 2 * N_HW * 8], F32, tag="bias")
        nc.sync.dma_start(bias_sb[:], bias_ap)

        # persistent activation tiles per direction, [128, 2048] per feature
        # chunk (both direction chains live concurrently for PE gap-filling)
        A = [[act_pool.tile([128, L], BF16, tag=f"A{d}{c}", name=f"A{d}{c}")
              for c in range(ND)] for d in (0, 1)]
        Bt = [[act_pool.tile([128, L], BF16, tag=f"B{d}{c}", name=f"B{d}{c}")
               for c in range(ND)] for d in (0, 1)]
        C = [[act_pool.tile([128, L], BF16, tag=f"C{d}{c}", name=f"C{d}{c}")
              for c in range(ND)] for d in (0, 1)]
        # natural-layout activations, dc-major so each chunk's xbar-transpose
        # destination is per-partition contiguous: zna[p, dc*L + k*128 + f]
        # = z_nat[k*128 + p, dc*128 + f]
        zna = [znat_pool.tile([128, ND * L], BF16, tag=f"zna{d}", name=f"zna{d}")
               for d in (0, 1)]
        # x resident in natural layout: layer-0 band lhsT for both directions
        xnat = [xnat_pool.tile([128, H], BF16, tag=f"xn{k}", name=f"xn{k}")
                for k in range(NT)]
        for k in range(NT):
            nc.sync.dma_start(xnat[k][:], x_ap[k * 128:(k + 1) * 128, :])

        def band(layer, dir_, dst):
            """dst: list of 4 chunk tiles [128, 2048] receiving y^T."""
            bsrc = bandf_ap if dir_ == 0 else bandb_ap
            bsb = band_pool.tile([128, 1536], BF16, tag="bandblk")
            nc.sync.dma_start(bsb[:], bsrc[layer])
            Pad = bsb[:, 0:1024]
            Spc = bsb[:, 1024:1536]
            members = _fwd_members if dir_ == 0 else _bwd_members
            if layer == 0:
                def src_ap(i, dc):
                    return xnat[i][:, dc * 128:(dc + 1) * 128]
            else:
                zd = zna[dir_]

                def src_ap(i, dc):
                    return zd[:, dc * L + i * 128:dc * L + (i + 1) * 128]
            for jp in range(NJ // 2):
                for dc in range(ND):
                    # [128,1024] PSUM tile spans 2 banks = 2 adjacent t-blocks;
                    # one evacuation instruction covers both
                    ps = mm_psum.tile([128, 1024], F32, tag="mm")
                    for half in (0, 1):
                        j = 2 * jp + half
                        mem = members(j, Pad, Spc)
                        n = len(mem)
                        for k, (i, rhs, lo, hi) in enumerate(mem):
                            nc.tensor.matmul(
                                ps[:, half * 512 + lo:half * 512 + hi],
                                src_ap(i, dc),
                                rhs,
                                start=(k == 0),
                                stop=(k == n - 1),
                            )
                    dstap = dst[dc][:, jp * 1024:(jp + 1) * 1024]
                    nc.scalar.copy(dstap, ps[:])

        def highway(layer, dir_, h, v, dst, resid=None):
            """v: input chunk tiles (transposed layout); dst: output chunk tiles.
            z = r + g*(v - r), r = relu(nl + bn), g = sigmoid(gt + bg)."""
            wts = []
            for dc in range(ND):
                wtile = wt_pool.tile([128, TWO_H], BF16, tag="wt")
                nc.sync.dma_start(
                    wtile[:], wt_ap[layer, dir_, h, dc * 128:(dc + 1) * 128, :])
                wts.append(wtile)
            bcol0 = ((layer * 2 + dir_) * N_HW + h) * 8
            # jp outer: all 4 chunks' half-row jp finish before jp+1 starts, so
            # the next stage's accumulation matmuls (which need every chunk)
            # unblock after half the gating instead of all of it
            for jp in range(NJ // 2):
                sl = slice(jp * 1024, (jp + 1) * 1024)
                for c in range(ND):
                    r = rg_pool.tile([128, 1024], BF16, tag="r")
                    g = rg_pool.tile([128, 1024], BF16, tag="g")
                    for part, oc in ((0, c), (1, 4 + c)):
                        ps = mm_psum.tile([128, 1024], F32, tag="mm")
                        for half in (0, 1):
                            j = 2 * jp + half
                            for dc in range(ND):
                                nc.tensor.matmul(
                                    ps[:, half * 512:(half + 1) * 512],
                                    wts[dc][:, oc * 128:(oc + 1) * 128],
                                    v[dc][:, j * 512:(j + 1) * 512],
                                    start=(dc == 0),
                                    stop=(dc == ND - 1),
                                )
                        bap = bias_sb[:, bcol0 + oc:bcol0 + oc + 1]
                        tgt = (r if part == 0 else g)[:]
                        if part == 0:
                            # r = relu(psum + bias); alternate engines by chunk
                            # parity so the gating-feeding DVE queue doesn't
                            # back up at stage tails (ACT has headroom)
                            if c % 2 == 0:
                                nc.vector.tensor_scalar(
                                    tgt, ps[:], bap, 0.0,
                                    mybir.AluOpType.add, mybir.AluOpType.max)
                            else:
                                nc.scalar.activation(
                                    tgt, ps[:],
                                    mybir.ActivationFunctionType.Relu, bias=bap)
                        else:
                            nc.scalar.activation(
                                tgt, ps[:],
                                mybir.ActivationFunctionType.Sigmoid, bias=bap)
                    # gate this half-row immediately: z = r + g*(v - r)
                    tmp = rg_pool.tile([128, 1024], BF16, tag="tmp")
                    nc.vector.tensor_sub(tmp[:], v[c][:, sl], r[:])
                    nc.vector.tensor_mul(tmp[:], tmp[:], g[:])
                    nc.vector.tensor_add(dst[c][:, sl], tmp[:], r[:])
                    if resid is not None:
                        nc.vector.tensor_add(
                            dst[c][:, sl], dst[c][:, sl], resid[c][:, sl])

        def transpose_out(layer, dir_, src, keep_nat):
            """src: 4 chunk tiles [128,2048] (transposed). xbar-DMA-transposes
            each chunk into zna[dir_] (natural layout, dc-major), then DMAs the
            natural data to DRAM out[layer, :, dir_*H:(dir_+1)*H]. zna doubles
            as the next layer's band input (layer-1 overwrite is WAR-safe)."""
            zd = zna[dir_]
            dram3 = out_ap[layer].rearrange("(k p) f -> p k f", p=128)
            # split per (chunk, half-row): each piece's gating finishes early
            # under the jp-outer highway loop, so these DMAs overlap compute
            for hb in range(2):
                for dc in range(ND):
                    off = dc * L + hb * 1024
                    dst3 = zd[:, off:off + 1024].rearrange(
                        "p (k f) -> p k f", f=128)
                    nc.sync.dma_start_transpose(
                        dst3, src[dc][:, hb * 1024:(hb + 1) * 1024])
                    nc.sync.dma_start(
                        dram3[:, hb * 8:(hb + 1) * 8,
                              dir_ * H + dc * 128:dir_ * H + (dc + 1) * 128],
                        dst3)

        # emit both directions interleaved per stage: the two chains are
        # independent, so whenever one stalls on its gating/evac tail the
        # scheduler has ready matmuls from the other to keep PE warm
        for d in (0, 1):
            band(0, d, A[d])
        for d in (0, 1):
            highway(0, d, 0, A[d], Bt[d])
        for d in (0, 1):
            highway(0, d, 1, Bt[d], C[d])
        for d in (0, 1):
            transpose_out(0, d, C[d], keep_nat=True)
        for d in (0, 1):
            band(1, d, A[d])
        for d in (0, 1):
            highway(1, d, 0, A[d], Bt[d])
        for d in (0, 1):
            highway(1, d, 1, Bt[d], A[d], resid=C[d])
        for d in (0, 1):
            transpose_out(1, d, A[d], keep_nat=False)

    nc.compile()
    return nc


_NC_CACHE = None
LAST_RESULTS = None


def _get_nc():
    global _NC_CACHE
    if _NC_CACHE is None:
        _NC_CACHE = build_nc()
    return _NC_CACHE


def kernel(inputs, masks, f_scores, b_scores, fW, fb, bW, bb):
    global LAST_RESULTS
    from concourse.bass_utils import run_bass_kernel_spmd

    in_maps = build_in_maps(inputs, f_scores, b_scores, fW, fb, bW, bb)
    nc = _get_nc()
    res = run_bass_kernel_spmd(nc, in_maps, core_ids=list(range(B)),
                               trace=bool(os.environ.get("BASS_TRACE")))
    LAST_RESULTS = res
    out = np.stack([res.results[b]["out"].astype(np.float32) for b in range(B)],
                   axis=1)
    return out
